# revision 2
# baseline (speedup 1.0000x reference)
"""Vocab-parallel softmax(x @ A.T) on 8 TRN2 NeuronCores.

Problem: input x [32, 1024] f32, atom_matrix A [128000, 1024] f32.
Output: softmax(x @ A.T, axis=-1) [32, 128000] f32.

Strategy (memory-bound: A is 512 MB):
  - Shard A row-wise (vocab dim) -> 16000 atoms/core (64 MB/core).
  - Host pre-transposes each shard to A^T [1024, 16000] so the d
    (contraction) axis lands on SBUF partitions with contiguous DMA.
  - Per core: stream A^T in 2 MB chunks, matmul (x^T stationary) into
    PSUM [32, 500] logits, Exp via ScalarE activation with accum_out
    giving per-chunk partial sums, all in one pass.
  - AllGather the per-core [32] exp-sums (256 B), reduce locally,
    normalize by 1/S, DMA out. Logits are O(1) by construction
    (LOGIT_SCALE in the model), so max-subtraction is unnecessary in
    fp32: |logit| <~ 5, exp(logit) <= ~150, sums ~1e5 -- all fine.
"""

import numpy as np

BATCH = 32
D = 1024
N_ATOMS = 128000
N_CORES = 8
SHARD = N_ATOMS // N_CORES  # 16000
CHUNK = 500                 # atoms per PSUM tile (<=512 fp32 moving free dim)
NCH = SHARD // CHUNK        # 32 chunks
KT = D // 128               # 8 contraction tiles

_state = {}


def _build(repeat=1):
    import concourse.mybir as mybir
    import concourse.tile as tile
    from concourse import bacc

    f32 = mybir.dt.float32
    nc = bacc.Bacc("TRN2", target_bir_lowering=False, debug=False,
                   num_devices=N_CORES)
    xT = nc.dram_tensor("xT", [D, BATCH], f32, kind="ExternalInput").ap()
    at = nc.dram_tensor("at", [D, SHARD], f32, kind="ExternalInput").ap()
    out = nc.dram_tensor("out", [BATCH, SHARD], f32, kind="ExternalOutput").ap()

    with tile.TileContext(nc) as tc:
        with (
            tc.tile_pool(name="xp", bufs=1) as xpool,
            tc.tile_pool(name="apool", bufs=4) as apool,
            tc.tile_pool(name="pp", bufs=2, space="PSUM") as pspool,
            tc.tile_pool(name="bigp", bufs=1) as bigpool,
            tc.tile_pool(name="smallp", bufs=1) as smallpool,
            tc.tile_pool(name="outp", bufs=4) as outpool,
            tc.tile_pool(name="dramp", bufs=1, space="DRAM") as drampool,
        ):
            for rep in range(repeat):
                if rep:
                    tc.strict_bb_all_engine_barrier()
                # x^T tiled by contraction: SBUF [128, KT, 32]; k-tile k
                # holds x^T rows k*128..(k+1)*128 (partition p <-> k*128+p).
                xs = xpool.tile([128, KT, BATCH], f32, name="xs")
                nc.sync.dma_start(xs, xT.rearrange("(k p) b -> p k b", p=128))

                exp_buf = bigpool.tile([BATCH, SHARD], f32, name="exp_buf")
                sums = smallpool.tile([BATCH, NCH], f32, name="sums")

                at_v = at.rearrange("(k p) a -> p k a", p=128)
                for c in range(NCH):
                    a_t = apool.tile([128, KT, CHUNK], f32, name="a_t")
                    nc.sync.dma_start(a_t, at_v[:, :, c * CHUNK:(c + 1) * CHUNK])
                    ps = pspool.tile([BATCH, CHUNK], f32, name="ps")
                    for k in range(KT):
                        nc.tensor.matmul(ps, lhsT=xs[:, k, :], rhs=a_t[:, k, :],
                                         start=(k == 0), stop=(k == KT - 1))
                    # exp(logits) -> SBUF, plus per-partition partial sums
                    nc.scalar.activation(
                        exp_buf[:, c * CHUNK:(c + 1) * CHUNK], ps,
                        mybir.ActivationFunctionType.Exp,
                        accum_out=sums[:, c:c + 1])

                # Local sum over chunks -> [32, 1]
                lsum = smallpool.tile([BATCH, 1], f32, name="lsum")
                nc.vector.reduce_sum(lsum, sums, axis=mybir.AxisListType.X)

                # AllGather per-core sums (256 B), reduce locally.
                cc_in = drampool.tile([BATCH, 1], f32, name="cc_in")
                cc_out = drampool.tile([N_CORES, BATCH], f32,
                                       addr_space="Shared", name="cc_out")
                nc.sync.dma_start(cc_in, lsum)
                nc.gpsimd.collective_compute(
                    "AllGather", mybir.AluOpType.bypass,
                    replica_groups=[list(range(N_CORES))],
                    ins=[cc_in.opt()], outs=[cc_out.opt()])
                gat = smallpool.tile([BATCH, N_CORES], f32, name="gat")
                # transpose-on-read: partition b <- gathered[:, b]
                nc.sync.dma_start(gat, cc_out.rearrange("r b -> b r"))
                gsum = smallpool.tile([BATCH, 1], f32, name="gsum")
                nc.vector.reduce_sum(gsum, gat, axis=mybir.AxisListType.X)
                rinv = smallpool.tile([BATCH, 1], f32, name="rinv")
                nc.vector.reciprocal(rinv, gsum)

                # Normalize and store, sliced for DMA overlap; alternate
                # ScalarE / VectorE so both engines share the tail.
                NS = 8
                W = SHARD // NS
                for s in range(NS):
                    sl = slice(s * W, (s + 1) * W)
                    ot = outpool.tile([BATCH, W], f32, name="ot")
                    if s % 2 == 0:
                        nc.scalar.mul(ot, exp_buf[:, sl], rinv)
                    else:
                        nc.vector.tensor_scalar_mul(ot, exp_buf[:, sl], rinv)
                    nc.sync.dma_start(out[:, sl], ot)

    nc.compile()
    return nc


def _get_nc():
    if "nc" not in _state:
        _state["nc"] = _build()
    return _state["nc"]


def make_in_maps(input, atom_matrix):
    xT = np.ascontiguousarray(input.T.astype(np.float32, copy=False))
    in_maps = []
    for i in range(N_CORES):
        shard = atom_matrix[i * SHARD:(i + 1) * SHARD, :]
        at_i = np.ascontiguousarray(shard.T.astype(np.float32, copy=False))
        in_maps.append({"xT": xT, "at": at_i})
    return in_maps


def kernel(input, atom_matrix):
    from concourse import bass_utils

    nc = _get_nc()
    in_maps = make_in_maps(input, atom_matrix)
    res = bass_utils.run_bass_kernel_spmd(
        nc, in_maps, core_ids=list(range(N_CORES)))
    return np.concatenate(
        [res.results[i]["out"] for i in range(N_CORES)], axis=1)
